# revision 8
# baseline (speedup 1.0000x reference)
"""Trainium2 Bass kernel for nn_ContConv1dDenseSim (banded continuous conv).

Math (reference):
  dt[b,l,j] = times[b,l]-times[b,j], masked to a causal band j in [l-W+1, l]
  (W = (sim_size+1)*kernel_size = 30), true_ids[b,j], and a row-validity mask.
  h = relu(dt*w1+b1)  (8 hidden), kv = (h@w2+b2) masked, reshaped (16,16)
  out[b,l,o] = sum_{j,i} features[b,j,i] * kv[b,l,j,i,o]

Factorization:
  G_k[jl,o] = tiw[jl] * (f[jl] @ W2_k)   (k=0..7), plus two extra host-built
  G blocks that absorb the band mask algebraically (see below).
  A'_k[jl,p] = relu(w1k*dtm + b1k) where dtm = band (.) dt (masked once).
  Out-of-band, dtm=0 so A'_k = relu(b1k) = c_k exactly; the resulting garbage
  contribution c_k * sum_{jl not in band} G_k is cancelled by two correction
  channels whose weights the host computes from b1:
    band-channel  G block = f @ (B2 + sum_k c_k W2_k)  (moving rhs = band)
    ones-channel  G block = f @ (-sum_k c_k W2_k)      (moving rhs = ones)
  so no per-channel band multiplies are needed on device.

  outT[o,p] = rv[p] * sum_{jl,ch} Gch[jl,o] * Ach[jl,p]: the banded
  contraction runs with the SMALL G factor stationary (LDWEIGHTS of 16
  columns ~13ns) and the big A matrices moving (N=128), 20 PSUM-accumulated
  matmuls. Output is produced transposed (16,128); the host transposes back.

Sharding: 8 cores = 2 batches x 4 query-row blocks of 128. Each core sees a
157-column key window (128 + W-1), folded on SBUF partitions as
[jl=0..127 | jl=128..156 (+pad)] sharing partitions.

NOTE: TRN2 engine instructions only encode a single sync-wait, so the program
is ordered so each engine's first touch of any foreign-produced tensor is an
instruction with exactly one new cross-engine dependency (tiny "observer" ops
where needed), and the Tile kernel-tail drain is pre-satisfied by single-wait
SP nops.
"""

import numpy as np
import concourse.bass as bass
import concourse.tile as tile
import concourse.mybir as mybir
from concourse.bass_utils import run_bass_kernel_spmd
from concourse.tile_rust import add_dep_helper

F32 = mybir.dt.float32
F16 = mybir.dt.float16
Alu = mybir.AluOpType
Act = mybir.ActivationFunctionType

BS, L, CH, HID, KS = 2, 512, 16, 8, 5
LBLK = 128                      # query rows per core
NBLK = L // LBLK                # 4
NCORES = BS * NBLK              # 8
W2 = 2 * LBLK                   # folded window width (256)
NG = 10 * CH                    # G columns: 8 hidden + band-chan + ones-chan
NPAR = 2 + 2 * HID + LBLK       # packed per-partition params columns

# relu-channel engine split: per k in 0..7, which engine computes A'_k
#   'a' = Scalar/ACT (1 op), 'v' = DVE (2 ops), 'g' = GpSimd (2 ops)
RELU_ENG = "aaaavvvv"
N_WARMUP_MM = 0
# accumulation order of the 10 channels in the PSUM chain: 'O'=ones-chan,
# 'B'=band-chan, digits = relu channels (ordered by expected readiness)
MM_ORDER = ["O", "B", 0, 4, 1, 5, 2, 6, 3, 7]

# test harness hooks
TRACE = False
LAST = None

_prog_cache = {}


def _build(W):
    """Build the single-core SPMD program for band width W (30 for sim=5)."""
    WIN = LBLK + W - 1          # real window columns (157)
    LO = WIN - 128              # columns in the second fold (29)
    nc = bass.Bass(trn_type="TRN2")

    # row0: [ones(256) | t_row(128)], row1: [t_win folded, 0-pad (256) | -1]
    dtp = nc.declare_dram_parameter("dtp", [2, W2 + LBLK], F16, isOutput=False)
    # feat_winT folded, 0-pad (cols 0:256) | W2p 160 cols (cols 256:416)
    fwp = nc.declare_dram_parameter("fwp", [CH, W2 + NG], F16, isOutput=False)
    # col 0: tiw_up, col 1: tiw_lo (padded), cols 2:10 w1, 10:18 b1,
    # cols 18:146 rv broadcast (read on partitions 0:16)
    par = nc.declare_dram_parameter("par", [128, NPAR], F32, isOutput=False)
    out_d = nc.declare_dram_parameter("out", [CH, LBLK], F32, isOutput=True)

    with tile.TileContext(nc) as tc:
        with (
            tc.tile_pool(name="sb", bufs=1) as sb,
            tc.tile_pool(name="ps", bufs=1, space="PSUM") as ps,
        ):
            # ---- input loads: two issuing sequencers ----
            t_dtp = sb.tile([2, W2 + LBLK], F16)
            dma_a = nc.sync.dma_start(t_dtp[:], dtp[:])
            t_fwp = sb.tile([CH, W2 + NG], F16)
            dma_b = nc.scalar.dma_start(t_fwp[:], fwp[:])
            sjunk = None
            if N_WARMUP_MM:
                sjunk = sb.tile([128, 512], F16)
                nc.gpsimd.memset(sjunk[:], 0.25)
            t_par = sb.tile([128, NPAR], F32)
            dma_c = nc.gpsimd.dma_start(t_par[:], par[:])
            tiw_up = t_par[:, 0:1]
            tiw_lo = t_par[:, 1:2]
            w1c = lambda k: t_par[:, 2 + k:3 + k]
            b1c = lambda k: t_par[:, 2 + HID + k:3 + HID + k]
            rv16 = t_par[0:CH, 2 + 2 * HID:2 + 2 * HID + LBLK]

            # ---- band mask, folded: [:,0:128] up block, [:,128:256] lo ----
            ones = sb.tile([128, W2], F16)
            nc.vector.memset(ones[:], 1.0)
            btmp = sb.tile([128, W2], F16)
            band = sb.tile([128, W2], F16)
            # up: keep jl - p >= 0  (jl = q)
            nc.gpsimd.affine_select(btmp[:, 0:LBLK], ones[:, 0:LBLK],
                                    [[-1, LBLK]], Alu.is_ge, 0.0,
                                    base=0, channel_multiplier=1)
            # up: keep (W-1) - jl + p >= 0
            nc.gpsimd.affine_select(band[:, 0:LBLK], btmp[:, 0:LBLK],
                                    [[1, LBLK]], Alu.is_ge, 0.0,
                                    base=W - 1, channel_multiplier=-1)
            # lo (jl = 128+q): keep p - (128-(W-1)) - q >= 0
            nc.gpsimd.affine_select(btmp[:, LBLK:W2], ones[:, LBLK:W2],
                                    [[1, LBLK]], Alu.is_ge, 0.0,
                                    base=(W - 1) - 128, channel_multiplier=-1)
            # lo: keep (LO-1) - q >= 0  (zero the fold's padding rows)
            last_gp_sel = nc.gpsimd.affine_select(band[:, LBLK:W2],
                                                  btmp[:, LBLK:W2],
                                                  [[0, LBLK]], Alu.is_ge, 0.0,
                                                  base=LO - 1,
                                                  channel_multiplier=-1)

            # ---- PE HAM warm-up: junk matmuls on scratch data so the
            # tensor engine is at full clock when the real chain arrives ----
            last_junk = None
            if N_WARMUP_MM:
                p_junk = ps.tile([128, 512], F32)
                for _ in range(N_WARMUP_MM):
                    last_junk = nc.tensor.matmul(p_junk[:], sjunk[:, 0:LBLK],
                                                 sjunk[:], start=True,
                                                 stop=True)

            # ---- dtT[jl, p] = t_row[p] - t_win[jl], folded (128,256) ----
            p_dt = ps.tile([128, W2], F32)
            rhs_dt = t_dtp[:, W2:W2 + LBLK]
            mm_dt0 = nc.tensor.matmul(p_dt[:, 0:LBLK], t_dtp[:, 0:LBLK], rhs_dt,
                             start=True, stop=True)
            if last_junk is not None:
                add_dep_helper(mm_dt0.ins, last_junk.ins, sync=False,
                               reason="order real PE work after warm-up")
            nc.tensor.matmul(p_dt[:, LBLK:W2], t_dtp[:, LBLK:W2], rhs_dt,
                             start=True, stop=True)

            # ---- G[jl, c] = feat_win[jl] @ W2p, folded (128, 320) ----
            p_g = ps.tile([128, 2 * NG], F32)
            w2p_s = t_fwp[:, W2:W2 + NG]
            nc.tensor.matmul(p_g[:, 0:NG], t_fwp[:, 0:LBLK],
                             w2p_s, start=True, stop=True)
            nc.tensor.matmul(p_g[0:LO, NG:2 * NG], t_fwp[:, LBLK:LBLK + LO],
                             w2p_s, start=True, stop=True)

            # ---- observers (single-wait discipline, see module docstring) --
            obs_a = sb.tile([1, 1], F32)
            nc.scalar.activation(obs_a[:], t_par[0:1, 0:1], Act.Copy)
            obs_v = sb.tile([1, 2], F32)
            nc.vector.tensor_copy(obs_v[:, 0:1], t_par[0:1, 0:1])
            nc.vector.tensor_copy(obs_v[:, 1:2], band[0:1, LBLK:LBLK + 1])

            # ---- dtm = band (.) dt  (one masked-dt tensor, DVE) ----
            dtm = sb.tile([128, W2], F16)
            nc.vector.tensor_mul(dtm[:], p_dt[:], band[:])

            # ---- g_sb = tiw (.) G  (per-partition scalar, DVE) ----
            g_sb = sb.tile([128, 2 * NG], F16)
            nc.vector.tensor_scalar_mul(g_sb[:, 0:NG], p_g[:, 0:NG], tiw_up)
            nc.vector.tensor_scalar_mul(g_sb[0:LO, NG:2 * NG],
                                        p_g[0:LO, NG:2 * NG], tiw_lo[0:LO])

            # ---- A' channels: relu(w1k*dtm + b1k), one (128,256) per k ----
            a_full = sb.tile([128, HID * W2], F16)
            last_eng_op = {}
            for k in range(HID):
                a_k = a_full[:, k * W2:(k + 1) * W2]
                eng = RELU_ENG[k]
                if eng == "a":
                    op = nc.scalar.activation(a_k, dtm[:], Act.Relu,
                                              bias=b1c(k), scale=w1c(k))
                else:
                    e = nc.vector if eng == "v" else nc.gpsimd
                    e.tensor_scalar(a_k, dtm[:], w1c(k), b1c(k),
                                    Alu.mult, Alu.add)
                    op = e.tensor_scalar_max(a_k, a_k, 0.0)
                last_eng_op[eng] = op

            # ---- outT[o,p] = sum_{ch,jl} Gch[jl,o] * Ach[jl,p] ----
            # G slices are stationary (16-col LDWEIGHTS ~13ns); the big A
            # tensors are the moving side (N=128).
            p_out = ps.tile([CH, LBLK], F32)
            last_pe = None

            def mm_operands(ch, lo):
                if ch == "O":
                    gc0 = 9 * CH
                    rhs = ones
                elif ch == "B":
                    gc0 = 8 * CH
                    rhs = band
                else:
                    gc0 = ch * CH
                    rhs = a_full[:, ch * W2:(ch + 1) * W2]
                if lo:
                    return (g_sb[0:LO, NG + gc0:NG + gc0 + CH],
                            rhs[0:LO, LBLK:W2])
                return g_sb[:, gc0:gc0 + CH], rhs[:, 0:LBLK]

            n_mm = 2 * len(MM_ORDER)
            i_mm = 0
            for ch in MM_ORDER:
                for lo in (False, True):
                    lhsT, rhs = mm_operands(ch, lo)
                    last_pe = nc.tensor.matmul(
                        p_out[:], lhsT, rhs,
                        start=(i_mm == 0), stop=(i_mm == n_mm - 1))
                    i_mm += 1

            # ---- row-validity fold + store (transposed out) ----
            o_sb = sb.tile([CH, LBLK], F32)
            last_dve = nc.vector.tensor_mul(o_sb[:], p_out[:], rv16)
            dma_o = nc.sync.dma_start(out_d[:], o_sb[:])

            # The Tile kernel-tail drain waits on every outstanding
            # semaphore, but TRN2 instructions encode at most one sync
            # wait. Observe each producer from the SP sequencer with
            # single-wait nops so the drain itself needs none.
            producers = [dma_a, dma_b, dma_c, dma_o, last_gp_sel,
                         last_dve, last_pe]
            for eng in "avg":
                if eng in last_eng_op:
                    producers.append(last_eng_op[eng])
            for prod in producers:
                nop = nc.sync.nop(nofuse=True, hint="predrain_observer")
                add_dep_helper(nop.ins, prod.ins, sync=True,
                               reason="pre-drain single-wait observer")

    heavy = [(nm, type(i).__name__, len(i.sync_info.on_wait))
             for nm, i in nc.inst_map.items()
             if getattr(i, "sync_info", None) is not None
             and i.sync_info.on_wait
             and len(i.sync_info.on_wait) > 1
             and type(i).__name__ != "InstDrain"]
    if heavy:
        raise RuntimeError(f"multi-wait instructions would fail walrus: {heavy}")
    return nc


def kernel(times, features, lengths, true_ids, sim_size, w1, b1, w2, b2):
    global LAST
    times = np.ascontiguousarray(np.asarray(times, dtype=np.float32))
    features = np.ascontiguousarray(np.asarray(features, dtype=np.float32))
    lengths = np.asarray(lengths)
    true_ids = np.asarray(true_ids)
    sim = int(np.asarray(sim_size))
    w1 = np.asarray(w1, dtype=np.float32).reshape(-1)
    b1 = np.asarray(b1, dtype=np.float32).reshape(-1)
    w2 = np.asarray(w2, dtype=np.float32)
    b2 = np.asarray(b2, dtype=np.float32)

    W = (sim + 1) * KS
    WIN = LBLK + W - 1
    LO = WIN - 128
    assert 0 < LO <= 128

    if W not in _prog_cache:
        _prog_cache[W] = _build(W)
    nc = _prog_cache[W]

    # W2p blocks: 8 per-channel weights, then band-channel (B2 + sum c_k W2k),
    # then ones-channel (-sum c_k W2k) with c_k = relu(b1_k).
    w2k = w2.reshape(HID, CH, CH)            # [k, i, o]
    ck = np.float16(np.maximum(b1, 0.0)).astype(np.float32)  # [8], fp16-matched
    w2c = np.einsum("k,kio->io", ck, w2k)
    w2p = np.concatenate(
        [w2k.transpose(1, 0, 2).reshape(CH, HID * CH),
         b2.reshape(CH, CH) + w2c,
         -w2c], axis=1).astype(np.float32)   # [16, 160]

    in_maps = []
    for core in range(NCORES):
        b, blk = divmod(core, NBLK)
        l0 = blk * LBLK
        idx = np.arange(l0 - (W - 1), l0 + LBLK)
        valid = idx >= 0
        idxc = np.clip(idx, 0, L - 1)
        t_win = np.where(valid, times[b, idxc], 0.0).astype(np.float32)
        feat_win = np.where(valid[:, None], features[b, idxc, :], 0.0)
        tiw = (true_ids[b, idxc] & valid).astype(np.float32)
        t_row = times[b, l0:l0 + LBLK].astype(np.float32)
        rv = (np.arange(l0, l0 + LBLK) <=
              (sim + 1) * (int(lengths[b]) - 1)).astype(np.float32)

        dtp = np.zeros((2, W2 + LBLK), np.float16)
        dtp[0, :W2] = 1.0
        dtp[1, :WIN] = t_win
        dtp[0, W2:] = t_row
        dtp[1, W2:] = -1.0

        fwp = np.zeros((CH, W2 + NG), np.float16)
        fwp[:, :WIN] = feat_win.T
        fwp[:, W2:W2 + NG] = w2p

        par = np.zeros((128, NPAR), np.float32)
        par[:, 0] = tiw[:128]
        par[:LO, 1] = tiw[128:]
        par[:, 2:2 + HID] = w1[None, :]
        par[:, 2 + HID:2 + 2 * HID] = b1[None, :]
        par[:, 2 + 2 * HID:] = rv[None, :]
        in_maps.append({"dtp": dtp, "fwp": fwp, "par": par})

    res = run_bass_kernel_spmd(nc, in_maps, core_ids=list(range(NCORES)),
                               trace=TRACE)
    LAST = res

    out = np.zeros((BS, L, CH), np.float32)
    for core in range(NCORES):
        b, blk = divmod(core, NBLK)
        out[b, blk * LBLK:(blk + 1) * LBLK, :] = res.results[core]["out"].T
    return out


# revision 10
# speedup vs baseline: 1.0800x; 1.0800x over previous
"""Trainium2 Bass kernel for nn_ContConv1dDenseSim (banded continuous conv).

Math (reference):
  dt[b,l,j] = times[b,l]-times[b,j], masked to a causal band j in [l-W+1, l]
  (W = (sim_size+1)*kernel_size = 30), true_ids[b,j], and a row-validity mask.
  h = relu(dt*w1+b1)  (8 hidden), kv = (h@w2+b2) masked, reshaped (16,16)
  out[b,l,o] = sum_{j,i} features[b,j,i] * kv[b,l,j,i,o]

Factorization:
  G_k[jl,o] = tiw[jl] * (f[jl] @ W2_k)   (k=0..7), plus two extra host-built
  G blocks that absorb the band mask algebraically (see below).
  A'_k[jl,p] = relu(w1k*dtm + b1k) where dtm = band (.) dt (masked once).
  Out-of-band, dtm=0 so A'_k = relu(b1k) = c_k exactly; the resulting garbage
  contribution c_k * sum_{jl not in band} G_k is cancelled by two correction
  channels whose weights the host computes from b1:
    band-channel  G block = f @ (B2 + sum_k c_k W2_k)  (moving rhs = band)
    ones-channel  G block = f @ (-sum_k c_k W2_k)      (moving rhs = ones)
  so no per-channel band multiplies are needed on device.

  outT[o,p] = rv[p] * sum_{jl,ch} Gch[jl,o] * Ach[jl,p]: the banded
  contraction runs with the SMALL G factor stationary (LDWEIGHTS of 16
  columns ~13ns) and the big A matrices moving (N=128), 20 PSUM-accumulated
  matmuls. Output is produced transposed (16,128); the host transposes back.

Sharding: 8 cores = 2 batches x 4 query-row blocks of 128. Each core sees a
157-column key window (128 + W-1), folded on SBUF partitions as
[jl=0..127 | jl=128..156 (+pad)] sharing partitions.

NOTE: TRN2 engine instructions only encode a single sync-wait, so the program
is ordered so each engine's first touch of any foreign-produced tensor is an
instruction with exactly one new cross-engine dependency (tiny "observer" ops
where needed), and the Tile kernel-tail drain is pre-satisfied by single-wait
SP nops.
"""

import numpy as np
import concourse.bass as bass
import concourse.tile as tile
import concourse.mybir as mybir
from concourse.bass_utils import run_bass_kernel_spmd
from concourse.tile_rust import add_dep_helper

F32 = mybir.dt.float32
F16 = mybir.dt.float16
Alu = mybir.AluOpType
Act = mybir.ActivationFunctionType

BS, L, CH, HID, KS = 2, 512, 16, 8, 5
LBLK = 128                      # query rows per core
NBLK = L // LBLK                # 4
NCORES = BS * NBLK              # 8
W2 = 2 * LBLK                   # folded window width (256)
NG = 10 * CH                    # G columns: 8 hidden + band-chan + ones-chan
NPAR = 2 + 2 * HID + LBLK       # packed per-partition params columns

# relu-channel engine split: per k in 0..7, which engine computes A'_k
#   'a' = Scalar/ACT (1 op), 'v' = DVE (2 ops), 'g' = GpSimd (2 ops)
RELU_ENG = "aaaavvvv"
N_WARMUP_MM = 5
# accumulation order of the 10 channels in the PSUM chain: 'O'=ones-chan,
# 'B'=band-chan, digits = relu channels (ordered by expected readiness)
MM_ORDER = ["O", "B", 0, 4, 1, 5, 2, 6, 3, 7]

# test harness hooks
TRACE = False
LAST = None

_prog_cache = {}


def _build(W):
    """Build the single-core SPMD program for band width W (30 for sim=5)."""
    WIN = LBLK + W - 1          # real window columns (157)
    LO = WIN - 128              # columns in the second fold (29)
    nc = bass.Bass(trn_type="TRN2")

    # row0: [ones(256) | t_row(128)], row1: [t_win folded, 0-pad (256) | -1]
    dtp = nc.declare_dram_parameter("dtp", [2, W2 + LBLK], F16, isOutput=False)
    # feat_winT folded, 0-pad (cols 0:256) | W2p 160 cols (cols 256:416)
    fwp = nc.declare_dram_parameter("fwp", [CH, W2 + NG], F16, isOutput=False)
    # col 0: tiw_up, col 1: tiw_lo (padded), cols 2:10 w1, 10:18 b1,
    # cols 18:146 rv broadcast (read on partitions 0:16)
    par = nc.declare_dram_parameter("par", [128, NPAR], F32, isOutput=False)
    out_d = nc.declare_dram_parameter("out", [CH, LBLK], F32, isOutput=True)

    with tile.TileContext(nc) as tc:
        with (
            tc.tile_pool(name="sb", bufs=1) as sb,
            tc.tile_pool(name="ps", bufs=1, space="PSUM") as ps,
        ):
            # ---- input loads: two issuing sequencers ----
            t_dtp = sb.tile([2, W2 + LBLK], F16)
            dma_a = nc.sync.dma_start(t_dtp[:], dtp[:])
            t_fwp = sb.tile([CH, W2 + NG], F16)
            dma_b = nc.scalar.dma_start(t_fwp[:], fwp[:])
            sjunk = None
            if N_WARMUP_MM:
                sjunk = sb.tile([128, 512], F16)
                nc.gpsimd.memset(sjunk[:], 0.25)
            t_par = sb.tile([128, NPAR], F32)
            dma_c = nc.gpsimd.dma_start(t_par[:], par[:])
            tiw_up = t_par[:, 0:1]
            tiw_lo = t_par[:, 1:2]
            w1c = lambda k: t_par[:, 2 + k:3 + k]
            b1c = lambda k: t_par[:, 2 + HID + k:3 + HID + k]
            rv16 = t_par[0:CH, 2 + 2 * HID:2 + 2 * HID + LBLK]

            # ---- band mask, folded: [:,0:128] up block, [:,128:256] lo ----
            ones = sb.tile([128, W2], F16)
            nc.vector.memset(ones[:], 1.0)
            btmp = sb.tile([128, W2], F16)
            band = sb.tile([128, W2], F16)
            # up: keep jl - p >= 0  (jl = q)
            nc.gpsimd.affine_select(btmp[:, 0:LBLK], ones[:, 0:LBLK],
                                    [[-1, LBLK]], Alu.is_ge, 0.0,
                                    base=0, channel_multiplier=1)
            # up: keep (W-1) - jl + p >= 0
            nc.gpsimd.affine_select(band[:, 0:LBLK], btmp[:, 0:LBLK],
                                    [[1, LBLK]], Alu.is_ge, 0.0,
                                    base=W - 1, channel_multiplier=-1)
            # lo (jl = 128+q): keep p - (128-(W-1)) - q >= 0
            nc.gpsimd.affine_select(btmp[:, LBLK:W2], ones[:, LBLK:W2],
                                    [[1, LBLK]], Alu.is_ge, 0.0,
                                    base=(W - 1) - 128, channel_multiplier=-1)
            # lo: keep (LO-1) - q >= 0  (zero the fold's padding rows)
            last_gp_sel = nc.gpsimd.affine_select(band[:, LBLK:W2],
                                                  btmp[:, LBLK:W2],
                                                  [[0, LBLK]], Alu.is_ge, 0.0,
                                                  base=LO - 1,
                                                  channel_multiplier=-1)

            # ---- PE HAM warm-up: junk matmuls on scratch data so the
            # tensor engine is at full clock when the real chain arrives ----
            last_junk = None
            if N_WARMUP_MM:
                p_junk = ps.tile([128, 512], F32)
                for _ in range(N_WARMUP_MM):
                    last_junk = nc.tensor.matmul(p_junk[:],
                                                 sjunk[:, 0:LBLK],
                                                 sjunk[:], start=True,
                                                 stop=True)

            # ---- dtT[jl, p] = t_row[p] - t_win[jl], folded (128,256) ----
            p_dt = ps.tile([128, W2], F32)
            rhs_dt = t_dtp[:, W2:W2 + LBLK]
            mm_dt0 = nc.tensor.matmul(p_dt[:, 0:LBLK], t_dtp[:, 0:LBLK], rhs_dt,
                             start=True, stop=True)
            if last_junk is not None:
                add_dep_helper(mm_dt0.ins, last_junk.ins, sync=False,
                               reason="order real PE work after warm-up")
            nc.tensor.matmul(p_dt[:, LBLK:W2], t_dtp[:, LBLK:W2], rhs_dt,
                             start=True, stop=True)

            # ---- G[jl, c] = feat_win[jl] @ W2p, folded (128, 320) ----
            p_g = ps.tile([128, 2 * NG], F32)
            w2p_s = t_fwp[:, W2:W2 + NG]
            nc.tensor.matmul(p_g[:, 0:NG], t_fwp[:, 0:LBLK],
                             w2p_s, start=True, stop=True)
            nc.tensor.matmul(p_g[0:LO, NG:2 * NG], t_fwp[:, LBLK:LBLK + LO],
                             w2p_s, start=True, stop=True)

            # ---- observers (single-wait discipline, see module docstring) --
            obs_a = sb.tile([1, 1], F32)
            nc.scalar.activation(obs_a[:], t_par[0:1, 0:1], Act.Copy)
            obs_v = sb.tile([1, 2], F32)
            nc.vector.tensor_copy(obs_v[:, 0:1], t_par[0:1, 0:1])
            nc.vector.tensor_copy(obs_v[:, 1:2], band[0:1, LBLK:LBLK + 1])

            # ---- dtm = band (.) dt  (one masked-dt tensor, DVE) ----
            dtm = sb.tile([128, W2], F16)
            nc.vector.tensor_mul(dtm[:], p_dt[:], band[:])

            # ---- g_sb = tiw (.) G  (per-partition scalar, DVE) ----
            g_sb = sb.tile([128, 2 * NG], F16)
            nc.vector.tensor_scalar_mul(g_sb[:, 0:NG], p_g[:, 0:NG], tiw_up)
            nc.vector.tensor_scalar_mul(g_sb[0:LO, NG:2 * NG],
                                        p_g[0:LO, NG:2 * NG], tiw_lo[0:LO])

            # ---- A' channels: relu(w1k*dtm + b1k), one (128,256) per k ----
            a_full = sb.tile([128, HID * W2], F16)
            last_eng_op = {}
            for k in range(HID):
                a_k = a_full[:, k * W2:(k + 1) * W2]
                eng = RELU_ENG[k]
                if eng == "a":
                    op = nc.scalar.activation(a_k, dtm[:], Act.Relu,
                                              bias=b1c(k), scale=w1c(k))
                else:
                    e = nc.vector if eng == "v" else nc.gpsimd
                    e.tensor_scalar(a_k, dtm[:], w1c(k), b1c(k),
                                    Alu.mult, Alu.add)
                    op = e.tensor_scalar_max(a_k, a_k, 0.0)
                last_eng_op[eng] = op

            # ---- outT[o,p] = sum_{ch,jl} Gch[jl,o] * Ach[jl,p] ----
            # G slices are stationary (16-col LDWEIGHTS ~13ns); the big A
            # tensors are the moving side (N=128).
            p_out = ps.tile([CH, LBLK], F32)
            last_pe = None

            def mm_operands(ch, lo):
                if ch == "O":
                    gc0 = 9 * CH
                    rhs = ones
                elif ch == "B":
                    gc0 = 8 * CH
                    rhs = band
                else:
                    gc0 = ch * CH
                    rhs = a_full[:, ch * W2:(ch + 1) * W2]
                if lo:
                    return (g_sb[0:LO, NG + gc0:NG + gc0 + CH],
                            rhs[0:LO, LBLK:W2])
                return g_sb[:, gc0:gc0 + CH], rhs[:, 0:LBLK]

            n_mm = 2 * len(MM_ORDER)
            i_mm = 0
            for ch in MM_ORDER:
                for lo in (False, True):
                    lhsT, rhs = mm_operands(ch, lo)
                    last_pe = nc.tensor.matmul(
                        p_out[:], lhsT, rhs,
                        start=(i_mm == 0), stop=(i_mm == n_mm - 1))
                    i_mm += 1

            # ---- row-validity fold + store (transposed out) ----
            o_sb = sb.tile([CH, LBLK], F32)
            last_dve = nc.vector.tensor_mul(o_sb[:], p_out[:], rv16)
            dma_o = nc.sync.dma_start(out_d[:], o_sb[:])

            # The Tile kernel-tail drain waits on every outstanding
            # semaphore, but TRN2 instructions encode at most one sync
            # wait. Observe each producer from the SP sequencer with
            # single-wait nops so the drain itself needs none.
            producers = [dma_a, dma_b, dma_c, dma_o, last_gp_sel,
                         last_dve, last_pe]
            for eng in "avg":
                if eng in last_eng_op:
                    producers.append(last_eng_op[eng])
            for prod in producers:
                nop = nc.sync.nop(nofuse=True, hint="predrain_observer")
                add_dep_helper(nop.ins, prod.ins, sync=True,
                               reason="pre-drain single-wait observer")

    heavy = [(nm, type(i).__name__, len(i.sync_info.on_wait))
             for nm, i in nc.inst_map.items()
             if getattr(i, "sync_info", None) is not None
             and i.sync_info.on_wait
             and len(i.sync_info.on_wait) > 1
             and type(i).__name__ != "InstDrain"]
    if heavy:
        raise RuntimeError(f"multi-wait instructions would fail walrus: {heavy}")
    return nc


def kernel(times, features, lengths, true_ids, sim_size, w1, b1, w2, b2):
    global LAST
    times = np.ascontiguousarray(np.asarray(times, dtype=np.float32))
    features = np.ascontiguousarray(np.asarray(features, dtype=np.float32))
    lengths = np.asarray(lengths)
    true_ids = np.asarray(true_ids)
    sim = int(np.asarray(sim_size))
    w1 = np.asarray(w1, dtype=np.float32).reshape(-1)
    b1 = np.asarray(b1, dtype=np.float32).reshape(-1)
    w2 = np.asarray(w2, dtype=np.float32)
    b2 = np.asarray(b2, dtype=np.float32)

    W = (sim + 1) * KS
    WIN = LBLK + W - 1
    LO = WIN - 128
    assert 0 < LO <= 128

    if W not in _prog_cache:
        _prog_cache[W] = _build(W)
    nc = _prog_cache[W]

    # W2p blocks: 8 per-channel weights, then band-channel (B2 + sum c_k W2k),
    # then ones-channel (-sum c_k W2k) with c_k = relu(b1_k).
    w2k = w2.reshape(HID, CH, CH)            # [k, i, o]
    ck = np.float16(np.maximum(b1, 0.0)).astype(np.float32)  # [8], fp16-matched
    w2c = np.einsum("k,kio->io", ck, w2k)
    w2p = np.concatenate(
        [w2k.transpose(1, 0, 2).reshape(CH, HID * CH),
         b2.reshape(CH, CH) + w2c,
         -w2c], axis=1).astype(np.float32)   # [16, 160]

    in_maps = []
    for core in range(NCORES):
        b, blk = divmod(core, NBLK)
        l0 = blk * LBLK
        idx = np.arange(l0 - (W - 1), l0 + LBLK)
        valid = idx >= 0
        idxc = np.clip(idx, 0, L - 1)
        t_win = np.where(valid, times[b, idxc], 0.0).astype(np.float32)
        feat_win = np.where(valid[:, None], features[b, idxc, :], 0.0)
        tiw = (true_ids[b, idxc] & valid).astype(np.float32)
        t_row = times[b, l0:l0 + LBLK].astype(np.float32)
        rv = (np.arange(l0, l0 + LBLK) <=
              (sim + 1) * (int(lengths[b]) - 1)).astype(np.float32)

        dtp = np.zeros((2, W2 + LBLK), np.float16)
        dtp[0, :W2] = 1.0
        dtp[1, :WIN] = t_win
        dtp[0, W2:] = t_row
        dtp[1, W2:] = -1.0

        fwp = np.zeros((CH, W2 + NG), np.float16)
        fwp[:, :WIN] = feat_win.T
        fwp[:, W2:W2 + NG] = w2p

        par = np.zeros((128, NPAR), np.float32)
        par[:, 0] = tiw[:128]
        par[:LO, 1] = tiw[128:]
        par[:, 2:2 + HID] = w1[None, :]
        par[:, 2 + HID:2 + 2 * HID] = b1[None, :]
        par[:, 2 + 2 * HID:] = rv[None, :]
        in_maps.append({"dtp": dtp, "fwp": fwp, "par": par})

    res = run_bass_kernel_spmd(nc, in_maps, core_ids=list(range(NCORES)),
                               trace=TRACE)
    LAST = res

    out = np.zeros((BS, L, CH), np.float32)
    for core in range(NCORES):
        b, blk = divmod(core, NBLK)
        out[b, blk * LBLK:(blk + 1) * LBLK, :] = res.results[core]["out"].T
    return out
